# revision 23
# baseline (speedup 1.0000x reference)
"""L1-distance attention kernel for Trainium2 (8 NeuronCores, SPMD).

Problem: q, k: [B=2, T=512, H=8, D=64] fp32
         out[b,s,t,h] = -sum_d |q[b,s,h,d] - k[b,t,h,d]| / sqrt(D)

Sharding: 16 (b,h) pairs across 8 cores, 2 pairs per core, stacked in the
SBUF partition dim (pair0 -> partitions 0:64 holding d, pair1 -> 64:128).

Math: |q-k| = (q+k) - 2*min(q,k)  and  min(q,k) = q - relu(q-k), so
  -scale*sum_d|q-k| = 2*scale*sum_d min(k,q_s) - scale*K_t - scale*Q_s   (DVE)
                    = -2*scale*sum_d relu(q_s-k) - scale*K_t + scale*Q_s (ACT)
where Q_s = sum_d q[d,s], K_t = sum_d k[d,t].

v2: all-bf16 data path. q/k are bf16-rounded on host (rel err ~2e-3, well
inside the 2e-2 gate); producers are bf16 in/out (DVE 2x/4x perf modes);
matmuls are bf16 x bf16 -> fp32 PSUM (1 cycle/row instead of fp32's 4).
Selector weights are +-0.25 / -0.125 — exact in bf16, and bf16*bf16
products accumulate exactly in fp32, so the only error is the input
rounding. Matmuls are issued c-innermost so the 4 col-groups of the PE
array (tile_position=(0,32c)) run concurrently.

Per core, per query s (512 total):
  - producer: DVE tensor_scalar_min(k, q[:,s]) or ScalarE activation
    Relu(bias=q[:,s], scale=-1) -> [128, 512] bf16 tile, both pairs at once.
  - TensorE: one matmul per query with a host-built +-2*scale selector
    weight [128, 32] routing each (pair, query) sum to its own PSUM row
    (col-tiled); 16 queries accumulate per 32-row block, preceded by a
    -scale*K_t correction matmul (moving = k itself).
  - After 64 queries a [128, 512] PSUM tile is done -> copy to SBUF with the
    per-row +-scale*Q_s bias folded in -> DMA out.

Host builds selector weights / Q-sum biases and unscrambles output rows.
"""

import os

import numpy as np
import ml_dtypes

os.environ.setdefault("MYCRO_LOCAL_CACHE", "1")

BF16 = ml_dtypes.bfloat16

B, T, H, D = 2, 512, 8, 64
NCORES = 8
NGROUPS = 8  # query groups of 64 -> one PSUM tile each
SCALE = 1.0 / float(np.sqrt(np.float64(D)))  # 0.125

# Producer assignment per (c, jj) slot. HW-measured: DVE tensor_scalar_min
# bf16 ~105ns/op, ACT Relu ~520ns/op, so ACT takes ~10 of each group's 64
# slots (Bresenham-spread over the flattened (jj,c) index) and DVE the rest;
# ACT also does the 8 psum->sbuf bias copies (~570ns each).
ACT_PER_64 = 13


def slot_is_act(c, jj):
    i = 4 * jj + c
    return ((i + 1) * ACT_PER_64) // 64 - (i * ACT_PER_64) // 64 == 1

_cached = {}


def _build_module(reps=1):
    from concourse import bacc, tile
    import concourse.mybir as mybir

    f32 = mybir.dt.float32
    bf16 = mybir.dt.bfloat16
    nc = bacc.Bacc(
        "TRN2",
        target_bir_lowering=False,
        debug=False,
        enable_asserts=False,
        num_devices=1,
    )
    # q holds bf16-rounded values but is stored fp32: the DVE tensor_scalar
    # scalar operand must be fp32 (and scalar dtype doesn't gate perf modes).
    q_dram = nc.dram_tensor("q", [128, T], f32, kind="ExternalInput")
    k_dram = nc.dram_tensor("k", [128, T], bf16, kind="ExternalInput")
    w_dram = nc.dram_tensor("w", [128, 4, 16, 32], bf16, kind="ExternalInput")
    wk_dram = nc.dram_tensor("wk", [128, 128], bf16, kind="ExternalInput")
    qs_dram = nc.dram_tensor("qs", [128, NGROUPS], f32, kind="ExternalInput")
    out_dram = nc.dram_tensor("out", [NGROUPS, 128, T], f32, kind="ExternalOutput")

    with tile.TileContext(nc) as tc:
        with (
            tc.tile_pool(name="const", bufs=1) as cpool,
            tc.tile_pool(name="ad", bufs=16) as adpool,
            tc.tile_pool(name="osb", bufs=3) as opool,
            tc.tile_pool(name="psum", bufs=8, space="PSUM") as ppool,
        ):
            q_sb = cpool.tile([128, T], f32, tag="q")
            k_sb = cpool.tile([128, T], bf16, tag="k")
            w_sb = cpool.tile([128, 4, 16, 32], bf16, tag="w")
            wk_sb = cpool.tile([128, 128], bf16, tag="wk")
            qs_sb = cpool.tile([128, NGROUPS], f32, tag="qs")
            nc.sync.dma_start(q_sb[:], q_dram[:])
            nc.sync.dma_start(k_sb[:], k_dram[:])
            nc.sync.dma_start(w_sb[:], w_dram[:])
            nc.sync.dma_start(wk_sb[:], wk_dram[:])
            nc.sync.dma_start(qs_sb[:], qs_dram[:])

            def emit_rep():
                for g in range(NGROUPS):
                    emit_group(g)

            def emit_group(g):
                psum_t = ppool.tile([128, T], f32, tag="acc")
                # -scale * K_t correction for all 128 rows in one full-width
                # matmul; also starts the accumulation group for every
                # col-block at once.
                nc.tensor.matmul(
                    psum_t[:],
                    wk_sb[:],
                    k_sb[:],
                    start=True,
                    stop=False,
                    # col-groups run concurrent accumulation groups in
                    # disjoint partition ranges of one bank; the sim's
                    # zero-region bookkeeping is partition-base-blind and
                    # would flag this spuriously (HW has_written is
                    # per-element).
                    skip_group_check=True,
                )
                for jj in range(16):
                    ads = []
                    for c in range(4):
                        s = 64 * g + 16 * c + jj
                        ad = adpool.tile([128, T], bf16, tag="ad")
                        if slot_is_act(c, jj):
                            nc.scalar.activation(
                                ad[:],
                                k_sb[:],
                                mybir.ActivationFunctionType.Relu,
                                bias=q_sb[:, s : s + 1],
                                scale=-1.0,
                            )
                        else:
                            nc.vector.tensor_scalar_min(
                                ad[:], k_sb[:], q_sb[:, s : s + 1]
                            )
                        ads.append(ad)
                    for c in range(4):
                        nc.tensor.matmul(
                            psum_t[32 * c : 32 * c + 32, :],
                            w_sb[:, c, jj, :],
                            ads[c][:],
                            start=False,
                            stop=(jj == 15),
                            tile_position=(0, 32 * c),
                            skip_group_check=True,
                        )
                ob = opool.tile([128, T], f32, tag="ob")
                # copy + per-row bias (+-scale*Q_s) on ScalarE (DVE is the
                # busier engine)
                nc.scalar.activation(
                    ob[:],
                    psum_t[:],
                    mybir.ActivationFunctionType.Identity,
                    bias=qs_sb[:, g : g + 1],
                    scale=1.0,
                )
                nc.sync.dma_start(out_dram[g], ob[:])

            if reps == 1:
                emit_rep()
            else:
                # hardware loop: constant trip count, body = one full rep.
                # Same program size for any reps -> identical host cost,
                # used for delta-timing.
                with tc.For_i(0, reps):
                    emit_rep()

    nc.compile()
    return nc


def _host_weights():
    # w is per (c, jj): [128, 4, 16, 32] since the ACT/DVE sign choice
    # depends on (c, jj).
    w = np.zeros((128, 4, 16, 32), np.float32)
    for c in range(4):
        for jj in range(16):
            v = -2.0 * SCALE if slot_is_act(c, jj) else 2.0 * SCALE
            w[0:64, c, jj, 2 * jj] = v
            w[64:128, c, jj, 2 * jj + 1] = v
    wk = np.zeros((128, 128), np.float32)
    wk[0:64, 0::2] = -SCALE
    wk[64:128, 1::2] = -SCALE
    return w.astype(BF16), wk.astype(BF16)


def _host_qsum(qc):
    """qc: [128, T] per-core stacked q^T (bf16-rounded). Returns qs
    [128, NGROUPS] fp32: row r = 32c + 2jj + p of group g gets
    -+scale*sum_d q[pair p, d, s] with s = 64g + 16c + jj
    (sign + for ACT slots, - for DVE slots)."""
    qsum = qc.astype(np.float64).reshape(2, 64, T).sum(axis=1)  # [pair, s]
    qs = np.empty((128, NGROUPS), np.float64)
    for g in range(NGROUPS):
        for c in range(4):
            for jj in range(16):
                s = 64 * g + 16 * c + jj
                sign = 1.0 if slot_is_act(c, jj) else -1.0
                for p in range(2):
                    qs[32 * c + 2 * jj + p, g] = sign * SCALE * qsum[p, s]
    return qs.astype(np.float32)


def get_module(reps=1):
    key = ("nc", reps)
    nc = _cached.get(key)
    if nc is None:
        nc = _build_module(reps)
        _cached[key] = nc
    return nc


def make_in_maps(q, k):
    """Shard full [B,T,H,D] q/k into 8 per-core input maps."""
    q = np.asarray(q, dtype=np.float32).astype(BF16)
    k = np.asarray(k, dtype=np.float32).astype(BF16)
    # [B, T, H, D] -> [B, H, D, T] -> [B*H, D, T]
    qt = np.ascontiguousarray(q.transpose(0, 2, 3, 1)).reshape(B * H, D, T)
    kt = np.ascontiguousarray(k.transpose(0, 2, 3, 1)).reshape(B * H, D, T)
    w, wk = _host_weights()
    in_maps = []
    for c in range(NCORES):
        qc = np.ascontiguousarray(
            qt[2 * c : 2 * c + 2].reshape(128, T)
        ).astype(np.float32)
        kc = np.ascontiguousarray(kt[2 * c : 2 * c + 2].reshape(128, T))
        in_maps.append(
            {"q": qc, "k": kc, "w": w, "wk": wk, "qs": _host_qsum(qc)}
        )
    return in_maps


def assemble_output(core_outs):
    """core_outs: list of 8 arrays [NGROUPS, 128, T] -> full [B, T, T, H]."""
    outf = np.empty((B, T, T, H), np.float32)
    for c in range(NCORES):
        o = np.asarray(core_outs[c]).reshape(NGROUPS, 4, 16, 2, T)
        # row r = 32c + 2jj + p in group g  ->  query s = 64g + 16c + jj, pair p
        o = o.transpose(3, 0, 1, 2, 4).reshape(2, T, T)
        for p in range(2):
            pg = 2 * c + p
            b, h = divmod(pg, H)
            outf[b, :, :, h] = o[p]
    return outf


def _make_runner(nc):
    """Build a cached jitted SPMD runner for module `nc`.

    run_bass_kernel_spmd re-creates the jax.jit closure on every call, so
    each call pays a full retrace + relower of the whole module (hundreds of
    ms, scaling with instruction count). Hoisting the jit here makes repeat
    calls run at device speed. Returns out arrays concatenated on axis 0
    (shape [NCORES*128, ...]).
    """
    import jax
    import jax.numpy as jnp
    from jax.sharding import Mesh, PartitionSpec
    from jax.experimental.shard_map import shard_map
    import concourse.mybir as mybir
    from concourse import bass2jax

    bass2jax.install_neuronx_cc_hook()

    partition_name = (
        nc.partition_id_tensor.name if nc.partition_id_tensor else None
    )
    in_names = []
    out_names = []
    out_avals = []
    for alloc in nc.m.functions[0].allocations:
        if not isinstance(alloc, mybir.MemoryLocationSet):
            continue
        name = alloc.memorylocations[0].name
        if alloc.kind == "ExternalInput":
            if name != partition_name:
                in_names.append(name)
        elif alloc.kind == "ExternalOutput":
            out_names.append(name)
            out_avals.append(
                jax.core.ShapedArray(
                    tuple(alloc.tensor_shape), mybir.dt.np(alloc.dtype)
                )
            )
    n_params = len(in_names)
    n_outs = len(out_avals)
    all_names = in_names + out_names
    if partition_name is not None:
        all_names = all_names + [partition_name]

    def _body(*args):
        operands = list(args)
        if partition_name is not None:
            operands.append(bass2jax.partition_id_tensor())
        outs = bass2jax._bass_exec_p.bind(
            *operands,
            out_avals=tuple(out_avals),
            in_names=tuple(all_names),
            out_names=tuple(out_names),
            lowering_input_output_aliases=(),
            sim_require_finite=True,
            sim_require_nnan=True,
            nc=nc,
        )
        return tuple(outs)

    body_fn = _body
    out_specs_n = n_outs

    devices = jax.devices()[:NCORES]
    mesh = Mesh(np.asarray(devices), ("core",))
    in_specs = (PartitionSpec("core"),) * (n_params + n_outs)
    out_specs = (PartitionSpec("core"),) * out_specs_n
    jitted = jax.jit(
        shard_map(
            body_fn,
            mesh=mesh,
            in_specs=in_specs,
            out_specs=out_specs,
            check_rep=False,
        ),
        keep_unused=True,
    )

    def run(concat_args):
        return jitted(*concat_args)

    def device_args(concat_args):
        from jax.sharding import NamedSharding

        sh = NamedSharding(mesh, PartitionSpec("core"))
        return [jax.device_put(a, sh) for a in concat_args]

    run.in_names = in_names
    run.out_names = out_names
    run.out_avals = out_avals
    run.n_params = n_params
    run.device_args = device_args
    return run


def get_runner(reps=1):
    key = ("runner", reps)
    r = _cached.get(key)
    if r is None:
        r = _make_runner(get_module(reps))
        _cached[key] = r
    return r


def concat_inputs(runner, in_maps):
    """Concatenate per-core inputs on axis 0 + uninit output buffers."""
    concat_in = [
        np.concatenate([m[name] for m in in_maps], axis=0)
        for name in runner.in_names
    ]
    for av in runner.out_avals:
        concat_in.append(
            np.empty((NCORES * av.shape[0], *av.shape[1:]), av.dtype)
        )
    return concat_in


def kernel(q, k):
    runner = get_runner()
    in_maps = make_in_maps(q, k)
    outs = runner(concat_inputs(runner, in_maps))
    full = np.asarray(outs[0]).reshape(NCORES, NGROUPS, 128, T)
    return assemble_output(list(full))


# revision 33
# speedup vs baseline: 1.3392x; 1.3392x over previous
"""L1-distance attention kernel for Trainium2 (8 NeuronCores, SPMD).

Problem: q, k: [B=2, T=512, H=8, D=64] fp32
         out[b,s,t,h] = -sum_d |q[b,s,h,d] - k[b,t,h,d]| / sqrt(D)

Sharding: 16 (b,h) pairs across 8 cores, 2 pairs per core, stacked in the
SBUF partition dim (pair0 -> partitions 0:64 holding d, pair1 -> 64:128).

Math: |q-k| = (q+k) - 2*min(q,k)  and  min(q,k) = q - relu(q-k), so
  -scale*sum_d|q-k| = 2*scale*sum_d min(k,q_s) - scale*K_t - scale*Q_s   (DVE)
                    = -2*scale*sum_d relu(q_s-k) - scale*K_t + scale*Q_s (ACT)
where Q_s = sum_d q[d,s], K_t = sum_d k[d,t].

v2: all-bf16 data path. q/k are bf16-rounded on host (rel err ~2e-3, well
inside the 2e-2 gate); producers are bf16 in/out (DVE 2x/4x perf modes);
matmuls are bf16 x bf16 -> fp32 PSUM (1 cycle/row instead of fp32's 4).
Selector weights are +-0.25 / -0.125 — exact in bf16, and bf16*bf16
products accumulate exactly in fp32, so the only error is the input
rounding. Matmuls are issued c-innermost so the 4 col-groups of the PE
array (tile_position=(0,32c)) run concurrently.

Per core, per query s (512 total):
  - producer: DVE tensor_scalar_min(k, q[:,s]) or ScalarE activation
    Relu(bias=q[:,s], scale=-1) -> [128, 512] bf16 tile, both pairs at once.
  - TensorE: one matmul per query with a host-built +-2*scale selector
    weight [128, 32] routing each (pair, query) sum to its own PSUM row
    (col-tiled); 16 queries accumulate per 32-row block, preceded by a
    -scale*K_t correction matmul (moving = k itself).
  - After 64 queries a [128, 512] PSUM tile is done -> copy to SBUF with the
    per-row +-scale*Q_s bias folded in -> DMA out.

Host builds selector weights / Q-sum biases and unscrambles output rows.
"""

import os

import numpy as np
import ml_dtypes

os.environ.setdefault("MYCRO_LOCAL_CACHE", "1")

BF16 = ml_dtypes.bfloat16

B, T, H, D = 2, 512, 8, 64
NCORES = 8
NGROUPS = 8  # query groups of 64 -> one PSUM tile each
SCALE = 1.0 / float(np.sqrt(np.float64(D)))  # 0.125

# Producer assignment per (c, jj) slot. HW-measured: DVE tensor_scalar_min
# bf16 ~105ns/op, ACT Relu ~520ns/op, so ACT takes ~10 of each group's 64
# slots (Bresenham-spread over the flattened (jj,c) index) and DVE the rest;
# ACT also does the 8 psum->sbuf bias copies (~570ns each).
ACT_PER_64 = 13

# ablation flags (bench-only; breaking them breaks correctness)
EMIT_WK = True
EMIT_COPY = True
EMIT_DMA = True
AD_BUFS = 16
PIPE_AHEAD = 0  # waves of producers issued ahead of their matmuls


def slot_is_act(c, jj):
    i = 4 * jj + c
    return ((i + 1) * ACT_PER_64) // 64 - (i * ACT_PER_64) // 64 == 1

_cached = {}


def _build_module(reps=1):
    from concourse import bacc, tile
    import concourse.mybir as mybir

    f32 = mybir.dt.float32
    bf16 = mybir.dt.bfloat16
    nc = bacc.Bacc(
        "TRN2",
        target_bir_lowering=False,
        debug=False,
        enable_asserts=False,
        num_devices=1,
    )
    # q holds bf16-rounded values but is stored fp32: the DVE tensor_scalar
    # scalar operand must be fp32 (and scalar dtype doesn't gate perf modes).
    q_dram = nc.dram_tensor("q", [128, T], f32, kind="ExternalInput")
    k_dram = nc.dram_tensor("k", [128, T], bf16, kind="ExternalInput")
    w_dram = nc.dram_tensor("w", [128, 4, 16, 32], bf16, kind="ExternalInput")
    wk_dram = nc.dram_tensor("wk", [128, 128], bf16, kind="ExternalInput")
    qs_dram = nc.dram_tensor("qs", [128, NGROUPS], f32, kind="ExternalInput")
    out_dram = nc.dram_tensor("out", [NGROUPS, 128, T], f32, kind="ExternalOutput")

    with tile.TileContext(nc) as tc:
        with (
            tc.tile_pool(name="const", bufs=1) as cpool,
            tc.tile_pool(name="adD", bufs=AD_BUFS) as adpool_dve,
            tc.tile_pool(name="adA", bufs=6) as adpool_act,
            tc.tile_pool(name="osb", bufs=3) as opool,
            tc.tile_pool(name="psum", bufs=8, space="PSUM") as ppool,
        ):
            q_sb = cpool.tile([128, T], f32, tag="q")
            k_sb = cpool.tile([128, T], bf16, tag="k")
            w_sb = cpool.tile([128, 4, 16, 32], bf16, tag="w")
            wk_sb = cpool.tile([128, 128], bf16, tag="wk")
            qs_sb = cpool.tile([128, NGROUPS], f32, tag="qs")
            nc.sync.dma_start(q_sb[:], q_dram[:])
            nc.sync.dma_start(k_sb[:], k_dram[:])
            nc.sync.dma_start(w_sb[:], w_dram[:])
            nc.sync.dma_start(wk_sb[:], wk_dram[:])
            nc.sync.dma_start(qs_sb[:], qs_dram[:])

            def emit_rep():
                for g in range(NGROUPS):
                    emit_group(g)

            def emit_group(g):
                psum_t = ppool.tile([128, T], f32, tag="acc")
                # -scale * K_t correction for all 128 rows in one full-width
                # matmul; also starts the accumulation group for every
                # col-block at once.
                nc.tensor.matmul(
                    psum_t[:],
                    wk_sb[:],
                    k_sb[:],
                    start=True,
                    stop=False,
                    # col-groups run concurrent accumulation groups in
                    # disjoint partition ranges of one bank; the sim's
                    # zero-region bookkeeping is partition-base-blind and
                    # would flag this spuriously (HW has_written is
                    # per-element).
                    skip_group_check=True,
                )
                def emit_producers(jj):
                    ads = []
                    for c in range(4):
                        s = 64 * g + 16 * c + jj
                        # per-engine pools: same-engine WAW ordering is free
                        # (engine FIFO), so each producer needs only the
                        # embeddable WAR-vs-PE wait.
                        if slot_is_act(c, jj):
                            ad = adpool_act.tile([128, T], bf16, tag="adA")
                            nc.scalar.activation(
                                ad[:],
                                k_sb[:],
                                mybir.ActivationFunctionType.Relu,
                                bias=q_sb[:, s : s + 1],
                                scale=-1.0,
                            )
                        else:
                            ad = adpool_dve.tile([128, T], bf16, tag="adD")
                            nc.vector.tensor_scalar_min(
                                ad[:], k_sb[:], q_sb[:, s : s + 1]
                            )
                        ads.append(ad)
                    return ads

                def emit_mms(jj, ads):
                    for c in range(4):
                        nc.tensor.matmul(
                            psum_t[32 * c : 32 * c + 32, :],
                            w_sb[:, c, jj, :],
                            ads[c][:],
                            start=False,
                            stop=(jj == 15),
                            tile_position=(0, 32 * c),
                            skip_group_check=True,
                        )

                # software-pipeline: producers run PIPE_AHEAD waves ahead of
                # their consuming matmuls in issue order.
                pend = []
                for jj in range(16):
                    pend.append((jj, emit_producers(jj)))
                    if len(pend) > PIPE_AHEAD:
                        j0, a0 = pend.pop(0)
                        emit_mms(j0, a0)
                for j0, a0 in pend:
                    emit_mms(j0, a0)
                if not EMIT_COPY:
                    return
                ob = opool.tile([128, T], f32, tag="ob")
                # copy + per-row bias (+-scale*Q_s) on ScalarE (DVE is the
                # busier engine)
                nc.scalar.activation(
                    ob[:],
                    psum_t[:],
                    mybir.ActivationFunctionType.Identity,
                    bias=qs_sb[:, g : g + 1],
                    scale=1.0,
                )
                if EMIT_DMA:
                    nc.sync.dma_start(out_dram[g], ob[:])

            if reps == 1:
                emit_rep()
            elif reps < 0:
                # unrolled repeats (bench-only): cross-iteration pipelining
                # preserved, but program size scales with |reps|.
                for _ in range(-reps):
                    emit_rep()
            else:
                # hardware loop: constant trip count, body = one full rep.
                # Same program size for any reps -> identical host cost,
                # used for delta-timing.
                with tc.For_i(0, reps):
                    emit_rep()

    nc.compile()
    return nc


def _host_weights():
    # w is per (c, jj): [128, 4, 16, 32] since the ACT/DVE sign choice
    # depends on (c, jj).
    w = np.zeros((128, 4, 16, 32), np.float32)
    for c in range(4):
        for jj in range(16):
            v = -2.0 * SCALE if slot_is_act(c, jj) else 2.0 * SCALE
            w[0:64, c, jj, 2 * jj] = v
            w[64:128, c, jj, 2 * jj + 1] = v
    wk = np.zeros((128, 128), np.float32)
    wk[0:64, 0::2] = -SCALE
    wk[64:128, 1::2] = -SCALE
    return w.astype(BF16), wk.astype(BF16)


def _host_qsum(qc):
    """qc: [128, T] per-core stacked q^T (bf16-rounded). Returns qs
    [128, NGROUPS] fp32: row r = 32c + 2jj + p of group g gets
    -+scale*sum_d q[pair p, d, s] with s = 64g + 16c + jj
    (sign + for ACT slots, - for DVE slots)."""
    qsum = qc.astype(np.float64).reshape(2, 64, T).sum(axis=1)  # [pair, s]
    qs = np.empty((128, NGROUPS), np.float64)
    for g in range(NGROUPS):
        for c in range(4):
            for jj in range(16):
                s = 64 * g + 16 * c + jj
                sign = 1.0 if slot_is_act(c, jj) else -1.0
                for p in range(2):
                    qs[32 * c + 2 * jj + p, g] = sign * SCALE * qsum[p, s]
    return qs.astype(np.float32)


def get_module(reps=1):
    key = ("nc", reps)
    nc = _cached.get(key)
    if nc is None:
        nc = _build_module(reps)
        _cached[key] = nc
    return nc


def make_in_maps(q, k):
    """Shard full [B,T,H,D] q/k into 8 per-core input maps."""
    q = np.asarray(q, dtype=np.float32).astype(BF16)
    k = np.asarray(k, dtype=np.float32).astype(BF16)
    # [B, T, H, D] -> [B, H, D, T] -> [B*H, D, T]
    qt = np.ascontiguousarray(q.transpose(0, 2, 3, 1)).reshape(B * H, D, T)
    kt = np.ascontiguousarray(k.transpose(0, 2, 3, 1)).reshape(B * H, D, T)
    w, wk = _host_weights()
    in_maps = []
    for c in range(NCORES):
        qc = np.ascontiguousarray(
            qt[2 * c : 2 * c + 2].reshape(128, T)
        ).astype(np.float32)
        kc = np.ascontiguousarray(kt[2 * c : 2 * c + 2].reshape(128, T))
        in_maps.append(
            {"q": qc, "k": kc, "w": w, "wk": wk, "qs": _host_qsum(qc)}
        )
    return in_maps


def assemble_output(core_outs):
    """core_outs: list of 8 arrays [NGROUPS, 128, T] -> full [B, T, T, H]."""
    outf = np.empty((B, T, T, H), np.float32)
    for c in range(NCORES):
        o = np.asarray(core_outs[c]).reshape(NGROUPS, 4, 16, 2, T)
        # row r = 32c + 2jj + p in group g  ->  query s = 64g + 16c + jj, pair p
        o = o.transpose(3, 0, 1, 2, 4).reshape(2, T, T)
        for p in range(2):
            pg = 2 * c + p
            b, h = divmod(pg, H)
            outf[b, :, :, h] = o[p]
    return outf


def _make_runner(nc):
    """Build a cached jitted SPMD runner for module `nc`.

    run_bass_kernel_spmd re-creates the jax.jit closure on every call, so
    each call pays a full retrace + relower of the whole module (hundreds of
    ms, scaling with instruction count). Hoisting the jit here makes repeat
    calls run at device speed. Returns out arrays concatenated on axis 0
    (shape [NCORES*128, ...]).
    """
    import jax
    import jax.numpy as jnp
    from jax.sharding import Mesh, PartitionSpec
    from jax.experimental.shard_map import shard_map
    import concourse.mybir as mybir
    from concourse import bass2jax

    bass2jax.install_neuronx_cc_hook()

    partition_name = (
        nc.partition_id_tensor.name if nc.partition_id_tensor else None
    )
    in_names = []
    out_names = []
    out_avals = []
    for alloc in nc.m.functions[0].allocations:
        if not isinstance(alloc, mybir.MemoryLocationSet):
            continue
        name = alloc.memorylocations[0].name
        if alloc.kind == "ExternalInput":
            if name != partition_name:
                in_names.append(name)
        elif alloc.kind == "ExternalOutput":
            out_names.append(name)
            out_avals.append(
                jax.core.ShapedArray(
                    tuple(alloc.tensor_shape), mybir.dt.np(alloc.dtype)
                )
            )
    n_params = len(in_names)
    n_outs = len(out_avals)
    all_names = in_names + out_names
    if partition_name is not None:
        all_names = all_names + [partition_name]

    def _body(*args):
        operands = list(args)
        if partition_name is not None:
            operands.append(bass2jax.partition_id_tensor())
        outs = bass2jax._bass_exec_p.bind(
            *operands,
            out_avals=tuple(out_avals),
            in_names=tuple(all_names),
            out_names=tuple(out_names),
            lowering_input_output_aliases=(),
            sim_require_finite=True,
            sim_require_nnan=True,
            nc=nc,
        )
        return tuple(outs)

    body_fn = _body
    out_specs_n = n_outs

    devices = jax.devices()[:NCORES]
    mesh = Mesh(np.asarray(devices), ("core",))
    in_specs = (PartitionSpec("core"),) * (n_params + n_outs)
    out_specs = (PartitionSpec("core"),) * out_specs_n
    jitted = jax.jit(
        shard_map(
            body_fn,
            mesh=mesh,
            in_specs=in_specs,
            out_specs=out_specs,
            check_rep=False,
        ),
        keep_unused=True,
    )

    def run(concat_args):
        return jitted(*concat_args)

    def device_args(concat_args):
        from jax.sharding import NamedSharding

        sh = NamedSharding(mesh, PartitionSpec("core"))
        return [jax.device_put(a, sh) for a in concat_args]

    run.in_names = in_names
    run.out_names = out_names
    run.out_avals = out_avals
    run.n_params = n_params
    run.device_args = device_args
    return run


def get_runner(reps=1):
    key = ("runner", reps)
    r = _cached.get(key)
    if r is None:
        r = _make_runner(get_module(reps))
        _cached[key] = r
    return r


def concat_inputs(runner, in_maps):
    """Concatenate per-core inputs on axis 0 + uninit output buffers."""
    concat_in = [
        np.concatenate([m[name] for m in in_maps], axis=0)
        for name in runner.in_names
    ]
    for av in runner.out_avals:
        concat_in.append(
            np.empty((NCORES * av.shape[0], *av.shape[1:]), av.dtype)
        )
    return concat_in


def kernel(q, k):
    runner = get_runner()
    in_maps = make_in_maps(q, k)
    outs = runner(concat_inputs(runner, in_maps))
    full = np.asarray(outs[0]).reshape(NCORES, NGROUPS, 128, T)
    return assemble_output(list(full))
